# revision 15
# baseline (speedup 1.0000x reference)
"""Trainium2 Bass kernel for the location-sensitive attention module.

Math (per batch b):
    q    = query @ Wq                              # (D_att,)
    k    = E @ Wk                                  # (T, D_att)
    loc  = conv1d(aw) -> (F, T);  loc_a = Wloc^T @ (conv + conv_b)
         = sum_k aw_pad[t+k] * M[k, :] + cbias     # M = conv_w^T @ Wloc  (31, 128)
    e_t  = tanh(q + k_t + loc_t) . Wscore          # (T,)
    w    = softmax(e)                              # (T,)
    ctx  = (w @ E) @ Wv                            # (D_dec,)

Sharding: data-parallel over batch across 8 cores (32 batches each).

Per-core dataflow (per batch):
    DMA E_b [512, 1024] natural ->
    PE-transpose 128x128 blocks -> PSUM -> ACT/DVE evac -> E^T chunks [128d, 512t]
    k^T[a,t]  = sum_dchunks Wk_c^T @ E^T_c  (+ loc matmul, accumulated in PSUM)
    tanh via ACT with per-partition bias q+cbias -> energy matvec (PE)
    exp+sum via ACT (accum_out), unnormalized p kept; reciprocal folded in at end
    w^T via small PE transposes -> ctx[1,1024] = sum_t p_t E[t,:]  (PE)
    ctx^T via small PE transposes -> batched final projection (w/ Wv) at end.
"""

import numpy as np

import concourse.bacc as bacc
import concourse.bass as bass
import concourse.mybir as mybir
import concourse.tile as tile
from concourse import masks

f32r = mybir.dt.float32r
f32 = mybir.dt.float32
AF = mybir.ActivationFunctionType

N_CORES = 8
B, T, D_DEC, D_ENC, D_ATT = 256, 512, 512, 1024, 128
N_FILT, KW, PAD = 32, 31, 15
B_PC = B // N_CORES

NT = T // 128          # 4 t-chunks
ND = D_ENC // 128      # 8 d-chunks
NQ = D_DEC // 128      # 4 dec-chunks
N_EVAC_ACT = 5         # d-chunks 0..4 evacuated by ACT, rest by DVE


def build_nc(b_pc=B_PC):
    nc = bacc.Bacc(target_bir_lowering=False)

    # encoder input split into chunks: single >16MB buffers wedge the
    # axon PJRT transfer path, so keep each ExternalInput buffer small
    n_enc_chunks = max(1, b_pc // 4)
    enc_chunks = [
        nc.dram_tensor(f"encoder_output_{i}", [b_pc // n_enc_chunks, T, D_ENC],
                       f32r, kind="ExternalInput")
        for i in range(n_enc_chunks)
    ]
    enc_bpc = b_pc // n_enc_chunks
    query = nc.dram_tensor("query", [b_pc, D_DEC], f32r, kind="ExternalInput")
    aw = nc.dram_tensor("attention_weights", [b_pc, T], f32r, kind="ExternalInput")
    Wq = nc.dram_tensor("Wq", [D_DEC, D_ATT], f32r, kind="ExternalInput")
    Wk = nc.dram_tensor("Wk", [D_ENC, D_ATT], f32r, kind="ExternalInput")
    Wv = nc.dram_tensor("Wv", [D_ENC, D_DEC], f32r, kind="ExternalInput")
    Wloc = nc.dram_tensor("Wloc", [N_FILT, D_ATT], f32r, kind="ExternalInput")
    conv_w = nc.dram_tensor("conv_w", [N_FILT, 1, KW], f32r, kind="ExternalInput")
    conv_b = nc.dram_tensor("conv_b", [N_FILT], f32r, kind="ExternalInput")
    Wscore = nc.dram_tensor("Wscore", [D_ATT, 1], f32r, kind="ExternalInput")
    ctx_d = nc.dram_tensor("context", [b_pc, D_DEC], f32r, kind="ExternalOutput")
    neww_d = nc.dram_tensor("new_w", [b_pc, T], f32r, kind="ExternalOutput")

    with tile.TileContext(nc) as tc:
        with (
            tc.tile_pool(name="pw", bufs=1) as pw,            # persistent weights/state
            tc.tile_pool(name="pnat", bufs=5) as pnat,        # E natural tiles
            tc.tile_pool(name="pet", bufs=8) as pet,          # E^T chunks
            tc.tile_pool(name="ptanh", bufs=3) as ptanh,
            tc.tile_pool(name="pmisc", bufs=3) as pmisc,
            tc.tile_pool(name="pband", bufs=2) as pband,
            tc.tile_pool(name="pdram", bufs=1, space="DRAM") as pdram,
            tc.tile_pool(name="ps_tp", bufs=2, space="PSUM") as ps_tp,
            tc.tile_pool(name="ps_kl", bufs=2, space="PSUM") as ps_kl,
            tc.tile_pool(name="ps_e", bufs=1, space="PSUM") as ps_e,
            tc.tile_pool(name="ps_sm", bufs=1, space="PSUM") as ps_sm,
            tc.tile_pool(name="ps_ctx", bufs=2, space="PSUM") as ps_ctx,
        ):
            # ---------------- preamble ----------------
            idf = pw.tile([128, 128], f32)
            masks.make_identity(nc, idf[:])
            idr = pw.tile([128, 128], f32r)
            nc.scalar.copy(idr[:], idf[:])
            dum = pw.tile([1, 128], f32)
            nc.gpsimd.memset(dum[:], 0.0)

            # first encoder tiles before anything else so the PE starts early
            nat_tiles, band_tiles = {}, {}
            band_d = pdram.tile([b_pc, T + 2 * PAD], f32r)

            def issue_nat(b):
                if b >= b_pc or b in nat_tiles:
                    return
                e_nat = pnat.tile([128, NT, D_ENC], f32r)
                src_ap = enc_chunks[b // enc_bpc][b % enc_bpc]
                nc.sync.dma_start(e_nat[:], src_ap.rearrange("(t p) d -> p t d", p=128))
                nat_tiles[b] = e_nat

            def issue_loads(b):
                issue_nat(b)

            issue_nat(0)
            issue_nat(1)

            # padded attention_weights staged once through DRAM; the per-oct
            # band reads use an overlapping AP over the padded rows
            awp_s = pw.tile([b_pc, T + 2 * PAD], f32r)
            nc.vector.memset(awp_s[:].bitcast(mybir.dt.uint32), 0)
            nc.sync.dma_start(awp_s[:, PAD:PAD + T], aw[:])
            nc.sync.dma_start(band_d[:], awp_s[:])
            band_octs = {}
            _stride = T + 2 * PAD

            def issue_oct(i):
                if i * 8 >= b_pc or i in band_octs:
                    return
                n = min(8, b_pc - i * 8)
                boct = pband.tile([KW, 8, T], f32r, tag="boct")
                nc.sync.dma_start(
                    boct[:, :n, :],
                    bass.AP(band_d.tensor, band_d[:].offset + i * 8 * _stride,
                            [[1, KW], [_stride, n], [1, T]]),
                )
                band_octs[i] = boct

            issue_oct(0)

            # weight loads, ordered by first use
            Wk_s = pw.tile([128, ND, D_ATT], f32r)
            nc.sync.dma_start(Wk_s[:], Wk[:].rearrange("(c p) a -> p c a", p=128))
            query_s = pw.tile([b_pc, D_DEC], f32r)
            nc.sync.dma_start(query_s[:], query[:])
            Wq_s = pw.tile([128, NQ, D_ATT], f32r)
            nc.sync.dma_start(Wq_s[:], Wq[:].rearrange("(c p) a -> p c a", p=128))
            Wloc_s = pw.tile([N_FILT, D_ATT], f32r)
            nc.sync.dma_start(Wloc_s[:], Wloc[:])
            convw_s = pw.tile([N_FILT, KW], f32r)
            nc.sync.dma_start(convw_s[:], conv_w[:, 0, :])
            convb_s = pw.tile([N_FILT, 2], f32r)
            nc.vector.memset(convb_s[:].bitcast(mybir.dt.uint32), 0)
            nc.sync.dma_start(convb_s[:, 0:1], bass.AP(conv_b, 0, [[1, N_FILT], [1, 1]]))
            Wsc_s = pw.tile([D_ATT, 1], f32r)
            nc.sync.dma_start(Wsc_s[:], Wscore[:])

            Wv_s = pw.tile([128, ND, D_DEC], f32r)
            nc.sync.dma_start(Wv_s[:], Wv[:].rearrange("(c p) a -> p c a", p=128))

            # PE warmup: absorb gpsimd tick
            dum_ps = ps_sm.tile([128, 128], f32, tag="sm")
            nc.tensor.transpose(dum_ps[:, :1], dum[:], idf[:1, :1])

            # q^T: transpose query then project:  qT[a, b] = sum_dec Wq[dec, a] query[b, dec]^T
            qtr_ps = ps_sm.tile([128, NQ * b_pc], f32r, tag="sm")
            for c in range(NQ):
                nc.tensor.transpose(
                    qtr_ps[:, c * b_pc:(c + 1) * b_pc],
                    query_s[:, c * 128:(c + 1) * 128],
                    idr[:b_pc, :b_pc],
                )
            qT_s = pw.tile([128, NQ, b_pc], f32r)
            nc.scalar.copy(qT_s[:].rearrange("p c b -> p (c b)"), qtr_ps[:])
            qt_ps = ps_kl.tile([128, b_pc], f32, tag="kl")
            for c in range(NQ):
                nc.tensor.matmul(
                    qt_ps[:], Wq_s[:, c, :], qT_s[:, c, :],
                    start=(c == 0), stop=(c == NQ - 1),
                )

            # cbias^T[a] = sum_f Wloc[f, a] conv_b[f]
            cb_ps = ps_sm.tile([128, 2], f32, tag="sm")
            nc.tensor.matmul(cb_ps[:], Wloc_s[:], convb_s[:], start=True, stop=True)
            cb_s = pw.tile([128, 1], f32)
            nc.scalar.copy(cb_s[:], cb_ps[:, 0:1])

            # M[k, a] = sum_f conv_w[f, k] Wloc[f, a]
            mm_ps = ps_sm.tile([KW, D_ATT], f32, tag="sm")
            nc.tensor.matmul(mm_ps[:], convw_s[:], Wloc_s[:], start=True, stop=True)
            Mmat_s = pw.tile([KW, D_ATT], f32r)
            nc.scalar.copy(Mmat_s[:], mm_ps[:])

            # qcb[a, b] = qT + cbias  (tanh bias, per-partition over a)
            qcb = pw.tile([128, b_pc], f32)
            nc.vector.tensor_scalar_add(qcb[:], qt_ps[:], cb_s[:])

            # persistent state
            ctxT_all = pw.tile([128, ND, b_pc], f32r)

            # ---------------- main loop ----------------
            for b in range(b_pc):
                issue_loads(b + 2)
                if b % 8 == 2:
                    issue_oct(b // 8 + 1)
                e_nat = nat_tiles.pop(b)

                kl_ps = ps_kl.tile([128, T], f32, tag="kl")
                for c in range(ND):
                    tp_ps = ps_tp.tile([128, T], f32r, tag="tp")
                    for t in range(NT):
                        nc.tensor.transpose(
                            tp_ps[:, t * 128:(t + 1) * 128],
                            e_nat[:, t, c * 128:(c + 1) * 128],
                            idr[:],
                        )
                    et = pet.tile([128, T], f32r)
                    if c < N_EVAC_ACT:
                        nc.scalar.copy(et[:], tp_ps[:])
                    else:
                        nc.vector.tensor_copy(et[:], tp_ps[:])
                    nc.tensor.matmul(
                        kl_ps[:], Wk_s[:, c, :], et[:],
                        start=(c == 0), stop=False,
                    )
                nc.tensor.matmul(kl_ps[:], Mmat_s[:], band_octs[b // 8][:, b % 8, :], start=False, stop=True)

                # tanh(k + loc + (q + cbias))
                tanh_t = ptanh.tile([128, T], f32r)
                nc.scalar.activation(tanh_t[:], kl_ps[:], AF.Tanh, bias=qcb[:, b:b + 1])

                # energies -> PSUM [1, T] @ partition 0
                e_ps = ps_e.tile([1, T], f32, tag="e")
                nc.tensor.matmul(e_ps[:], Wsc_s[:], tanh_t[:], start=True, stop=True)

                # p = exp(e), s = sum(p); w = p / s
                p_row = pmisc.tile([1, T], f32r, tag="prow")
                s_row = pmisc.tile([1, 1], f32, tag="srow")
                nc.scalar.activation(p_row[:], e_ps[:], AF.Exp, accum_out=s_row[:])
                r_row = pmisc.tile([1, 1], f32, tag="rrow")
                nc.vector.reciprocal(r_row[:], s_row[:])
                w_row = pmisc.tile([1, T], f32r, tag="wrow")
                nc.vector.tensor_scalar_mul(w_row[:], p_row[:], r_row[:])
                nc.gpsimd.dma_start(neww_d[b:b + 1, :], w_row[:])

                # p^T chunks [128, NT] (unnormalized; 1/s folded into ctx evac)
                wt_ps = ps_sm.tile([128, NT], f32, tag="sm")
                for t in range(NT):
                    nc.tensor.transpose(
                        wt_ps[:, t:t + 1],
                        p_row[0:1, t * 128:(t + 1) * 128].bitcast(f32),
                        idf[:1, :1],
                    )
                wT_s = pmisc.tile([128, NT], f32r, tag="wT")
                nc.scalar.copy(wT_s[:], wt_ps[:])

                # ctx[1, D_ENC] = sum_t w_t E[t, :]
                ctx_ps0 = ps_ctx.tile([1, D_DEC], f32, tag="ctx")
                ctx_ps1 = ps_ctx.tile([1, D_DEC], f32, tag="ctx")
                for h, cps in enumerate((ctx_ps0, ctx_ps1)):
                    for t in range(NT):
                        nc.tensor.matmul(
                            cps[:],
                            wT_s[:, t:t + 1],
                            e_nat[:, t, h * D_DEC:(h + 1) * D_DEC],
                            start=(t == 0), stop=(t == NT - 1),
                        )
                ctx_s = pmisc.tile([1, 2, D_DEC], f32r, tag="ctxs")
                nc.vector.tensor_scalar_mul(ctx_s[:, 0, :], ctx_ps0[:], r_row[:])
                nc.vector.tensor_scalar_mul(ctx_s[:, 1, :], ctx_ps1[:], r_row[:])

                # ctx^T chunks into per-batch column
                ctT_ps = ps_sm.tile([128, ND], f32, tag="sm")
                for c in range(ND):
                    nc.tensor.transpose(
                        ctT_ps[:, c:c + 1],
                        ctx_s[0:1, c // (D_DEC // 128), (c % (D_DEC // 128)) * 128:(c % (D_DEC // 128) + 1) * 128].bitcast(f32),
                        idf[:1, :1],
                    )
                nc.scalar.copy(ctxT_all[:, :, b], ctT_ps[:])

            # ---------------- postamble ----------------
            fp_ps = ps_tp.tile([b_pc, D_DEC], f32, tag="tp")
            for c in range(ND):
                nc.tensor.matmul(
                    fp_ps[:], ctxT_all[:, c, :], Wv_s[:, c, :],
                    start=(c == 0), stop=(c == ND - 1),
                )
            ctx_out_s = pw.tile([b_pc, D_DEC], f32r)
            nc.scalar.copy(ctx_out_s[:], fp_ps[:])
            nc.sync.dma_start(ctx_d[:], ctx_out_s[:])

    nc.finalize()
    return nc


_NC_CACHE = {}


def _get_nc(b_pc):
    if b_pc not in _NC_CACHE:
        _NC_CACHE[b_pc] = build_nc(b_pc)
    return _NC_CACHE[b_pc]


def kernel(query, encoder_output, attention_weights, Wq, Wk, Wv, Wloc,
           conv_w, conv_b, Wscore, _trace=False, _trace_kwargs=None):
    from concourse.bass_utils import run_bass_kernel_spmd

    b_pc = B // N_CORES
    nc = _get_nc(b_pc)
    shared = {
        "Wq": np.asarray(Wq, dtype=np.float32),
        "Wk": np.asarray(Wk, dtype=np.float32),
        "Wv": np.asarray(Wv, dtype=np.float32),
        "Wloc": np.asarray(Wloc, dtype=np.float32),
        "conv_w": np.asarray(conv_w, dtype=np.float32),
        "conv_b": np.asarray(conv_b, dtype=np.float32),
        "Wscore": np.asarray(Wscore, dtype=np.float32),
    }
    query = np.asarray(query, dtype=np.float32)
    encoder_output = np.asarray(encoder_output, dtype=np.float32)
    attention_weights = np.asarray(attention_weights, dtype=np.float32)
    n_enc_chunks = max(1, b_pc // 4)
    enc_bpc = b_pc // n_enc_chunks
    in_maps = []
    for c in range(N_CORES):
        sl = slice(c * b_pc, (c + 1) * b_pc)
        m = {
            "query": query[sl],
            "attention_weights": attention_weights[sl],
            **shared,
        }
        for i in range(n_enc_chunks):
            lo = c * b_pc + i * enc_bpc
            m[f"encoder_output_{i}"] = encoder_output[lo:lo + enc_bpc]
        in_maps.append(m)
    kw = {}
    if _trace:
        kw = {"trace": True, **(_trace_kwargs or {})}
    res = run_bass_kernel_spmd(nc, in_maps, list(range(N_CORES)), **kw)
    ctx = np.concatenate([res.results[c]["context"] for c in range(N_CORES)], axis=0)
    neww = np.concatenate([res.results[c]["new_w"] for c in range(N_CORES)], axis=0)
    kernel._last_result = res
    return ctx, neww


# revision 17
# speedup vs baseline: 265.7111x; 265.7111x over previous
"""Trainium2 Bass kernel for the location-sensitive attention module.

Math (per batch b):
    q    = query @ Wq                              # (D_att,)
    k    = E @ Wk                                  # (T, D_att)
    loc  = conv1d(aw) -> (F, T);  loc_a = Wloc^T @ (conv + conv_b)
         = sum_k aw_pad[t+k] * M[k, :] + cbias     # M = conv_w^T @ Wloc  (31, 128)
    e_t  = tanh(q + k_t + loc_t) . Wscore          # (T,)
    w    = softmax(e)                              # (T,)
    ctx  = (w @ E) @ Wv                            # (D_dec,)

Sharding: data-parallel over batch across 8 cores (32 batches each).

Per-core dataflow (per batch):
    DMA E_b [512, 1024] natural ->
    PE-transpose 128x128 blocks -> PSUM -> ACT/DVE evac -> E^T chunks [128d, 512t]
    k^T[a,t]  = sum_dchunks Wk_c^T @ E^T_c  (+ loc matmul, accumulated in PSUM)
    tanh via ACT with per-partition bias q+cbias -> energy matvec (PE)
    exp+sum via ACT (accum_out), unnormalized p kept; reciprocal folded in at end
    w^T via small PE transposes -> ctx[1,1024] = sum_t p_t E[t,:]  (PE)
    ctx^T via small PE transposes -> batched final projection (w/ Wv) at end.
"""

import numpy as np

import concourse.bacc as bacc
import concourse.bass as bass
import concourse.mybir as mybir
import concourse.tile as tile
from concourse import masks

f32r = mybir.dt.float32r
f32 = mybir.dt.float32
AF = mybir.ActivationFunctionType

N_CORES = 8
B, T, D_DEC, D_ENC, D_ATT = 256, 512, 512, 1024, 128
N_FILT, KW, PAD = 32, 31, 15
B_PC = B // N_CORES

NT = T // 128          # 4 t-chunks
ND = D_ENC // 128      # 8 d-chunks
NQ = D_DEC // 128      # 4 dec-chunks
N_EVAC_ACT = 5         # d-chunks 0..4 evacuated by ACT, rest by DVE


def build_nc(b_pc=B_PC, bench_loops=1):
    nc = bacc.Bacc(target_bir_lowering=False)

    # encoder input split into chunks: single >16MB buffers wedge the
    # axon PJRT transfer path, so keep each ExternalInput buffer small
    n_enc_chunks = max(1, b_pc // 4)
    enc_chunks = [
        nc.dram_tensor(f"encoder_output_{i}", [b_pc // n_enc_chunks, T, D_ENC],
                       f32r, kind="ExternalInput")
        for i in range(n_enc_chunks)
    ]
    enc_bpc = b_pc // n_enc_chunks
    query = nc.dram_tensor("query", [b_pc, D_DEC], f32r, kind="ExternalInput")
    aw = nc.dram_tensor("attention_weights", [b_pc, T], f32r, kind="ExternalInput")
    Wq = nc.dram_tensor("Wq", [D_DEC, D_ATT], f32r, kind="ExternalInput")
    Wk = nc.dram_tensor("Wk", [D_ENC, D_ATT], f32r, kind="ExternalInput")
    Wv = nc.dram_tensor("Wv", [D_ENC, D_DEC], f32r, kind="ExternalInput")
    Wloc = nc.dram_tensor("Wloc", [N_FILT, D_ATT], f32r, kind="ExternalInput")
    conv_w = nc.dram_tensor("conv_w", [N_FILT, 1, KW], f32r, kind="ExternalInput")
    conv_b = nc.dram_tensor("conv_b", [N_FILT], f32r, kind="ExternalInput")
    Wscore = nc.dram_tensor("Wscore", [D_ATT, 1], f32r, kind="ExternalInput")
    ctx_d = nc.dram_tensor("context", [b_pc, D_DEC], f32r, kind="ExternalOutput")
    neww_d = nc.dram_tensor("new_w", [b_pc, T], f32r, kind="ExternalOutput")

    import contextlib

    with tile.TileContext(nc) as tc:
        loop_cm = tc.For_i(0, bench_loops, 1) if bench_loops > 1 else contextlib.nullcontext()
        with loop_cm:
          with (
            tc.tile_pool(name="pw", bufs=1) as pw,            # persistent weights/state
            tc.tile_pool(name="pnat", bufs=5) as pnat,        # E natural tiles
            tc.tile_pool(name="pet", bufs=8) as pet,          # E^T chunks
            tc.tile_pool(name="ptanh", bufs=3) as ptanh,
            tc.tile_pool(name="pmisc", bufs=3) as pmisc,
            tc.tile_pool(name="pband", bufs=2) as pband,
            tc.tile_pool(name="pdram", bufs=1, space="DRAM") as pdram,
            tc.tile_pool(name="ps_tp", bufs=2, space="PSUM") as ps_tp,
            tc.tile_pool(name="ps_kl", bufs=2, space="PSUM") as ps_kl,
            tc.tile_pool(name="ps_e", bufs=1, space="PSUM") as ps_e,
            tc.tile_pool(name="ps_sm", bufs=1, space="PSUM") as ps_sm,
            tc.tile_pool(name="ps_ctx", bufs=2, space="PSUM") as ps_ctx,
        ):
            # ---------------- preamble ----------------
            idf = pw.tile([128, 128], f32)
            masks.make_identity(nc, idf[:])
            idr = pw.tile([128, 128], f32r)
            nc.scalar.copy(idr[:], idf[:])
            dum = pw.tile([1, 128], f32)
            nc.gpsimd.memset(dum[:], 0.0)

            # first encoder tiles before anything else so the PE starts early
            nat_tiles, band_tiles = {}, {}
            band_d = pdram.tile([b_pc, T + 2 * PAD], f32r)

            def issue_nat(b):
                if b >= b_pc or b in nat_tiles:
                    return
                e_nat = pnat.tile([128, NT, D_ENC], f32r)
                src_ap = enc_chunks[b // enc_bpc][b % enc_bpc]
                nc.sync.dma_start(e_nat[:], src_ap.rearrange("(t p) d -> p t d", p=128))
                nat_tiles[b] = e_nat

            def issue_loads(b):
                issue_nat(b)

            issue_nat(0)
            issue_nat(1)

            # padded attention_weights staged once through DRAM; the per-oct
            # band reads use an overlapping AP over the padded rows
            awp_s = pw.tile([b_pc, T + 2 * PAD], f32r)
            nc.vector.memset(awp_s[:].bitcast(mybir.dt.uint32), 0)
            nc.sync.dma_start(awp_s[:, PAD:PAD + T], aw[:])
            nc.sync.dma_start(band_d[:], awp_s[:])
            band_octs = {}
            _stride = T + 2 * PAD

            def issue_oct(i):
                if i * 8 >= b_pc or i in band_octs:
                    return
                n = min(8, b_pc - i * 8)
                boct = pband.tile([KW, 8, T], f32r, tag="boct")
                nc.sync.dma_start(
                    boct[:, :n, :],
                    bass.AP(band_d.tensor, band_d[:].offset + i * 8 * _stride,
                            [[1, KW], [_stride, n], [1, T]]),
                )
                band_octs[i] = boct

            issue_oct(0)

            # weight loads, ordered by first use
            Wk_s = pw.tile([128, ND, D_ATT], f32r)
            nc.sync.dma_start(Wk_s[:], Wk[:].rearrange("(c p) a -> p c a", p=128))
            query_s = pw.tile([b_pc, D_DEC], f32r)
            nc.sync.dma_start(query_s[:], query[:])
            Wq_s = pw.tile([128, NQ, D_ATT], f32r)
            nc.sync.dma_start(Wq_s[:], Wq[:].rearrange("(c p) a -> p c a", p=128))
            Wloc_s = pw.tile([N_FILT, D_ATT], f32r)
            nc.sync.dma_start(Wloc_s[:], Wloc[:])
            convw_s = pw.tile([N_FILT, KW], f32r)
            nc.sync.dma_start(convw_s[:], conv_w[:, 0, :])
            convb_s = pw.tile([N_FILT, 2], f32r)
            nc.vector.memset(convb_s[:].bitcast(mybir.dt.uint32), 0)
            nc.sync.dma_start(convb_s[:, 0:1], bass.AP(conv_b, 0, [[1, N_FILT], [1, 1]]))
            Wsc_s = pw.tile([D_ATT, 1], f32r)
            nc.sync.dma_start(Wsc_s[:], Wscore[:])

            Wv_s = pw.tile([128, ND, D_DEC], f32r)
            nc.sync.dma_start(Wv_s[:], Wv[:].rearrange("(c p) a -> p c a", p=128))

            # PE warmup: absorb gpsimd tick
            dum_ps = ps_sm.tile([128, 128], f32, tag="sm")
            nc.tensor.transpose(dum_ps[:, :1], dum[:], idf[:1, :1])

            # q^T: transpose query then project:  qT[a, b] = sum_dec Wq[dec, a] query[b, dec]^T
            qtr_ps = ps_sm.tile([128, NQ * b_pc], f32r, tag="sm")
            for c in range(NQ):
                nc.tensor.transpose(
                    qtr_ps[:, c * b_pc:(c + 1) * b_pc],
                    query_s[:, c * 128:(c + 1) * 128],
                    idr[:b_pc, :b_pc],
                )
            qT_s = pw.tile([128, NQ, b_pc], f32r)
            nc.scalar.copy(qT_s[:].rearrange("p c b -> p (c b)"), qtr_ps[:])
            qt_ps = ps_kl.tile([128, b_pc], f32, tag="kl")
            for c in range(NQ):
                nc.tensor.matmul(
                    qt_ps[:], Wq_s[:, c, :], qT_s[:, c, :],
                    start=(c == 0), stop=(c == NQ - 1),
                )

            # cbias^T[a] = sum_f Wloc[f, a] conv_b[f]
            cb_ps = ps_sm.tile([128, 2], f32, tag="sm")
            nc.tensor.matmul(cb_ps[:], Wloc_s[:], convb_s[:], start=True, stop=True)
            cb_s = pw.tile([128, 1], f32)
            nc.scalar.copy(cb_s[:], cb_ps[:, 0:1])

            # M[k, a] = sum_f conv_w[f, k] Wloc[f, a]
            mm_ps = ps_sm.tile([KW, D_ATT], f32, tag="sm")
            nc.tensor.matmul(mm_ps[:], convw_s[:], Wloc_s[:], start=True, stop=True)
            Mmat_s = pw.tile([KW, D_ATT], f32r)
            nc.scalar.copy(Mmat_s[:], mm_ps[:])

            # qcb[a, b] = qT + cbias  (tanh bias, per-partition over a)
            qcb = pw.tile([128, b_pc], f32)
            nc.vector.tensor_scalar_add(qcb[:], qt_ps[:], cb_s[:])

            # persistent state
            ctxT_all = pw.tile([128, ND, b_pc], f32r)

            # ---------------- main loop ----------------
            for b in range(b_pc):
                issue_loads(b + 2)
                if b % 8 == 2:
                    issue_oct(b // 8 + 1)
                e_nat = nat_tiles.pop(b)

                kl_ps = ps_kl.tile([128, T], f32, tag="kl")
                for c in range(ND):
                    tp_ps = ps_tp.tile([128, T], f32r, tag="tp")
                    for t in range(NT):
                        nc.tensor.transpose(
                            tp_ps[:, t * 128:(t + 1) * 128],
                            e_nat[:, t, c * 128:(c + 1) * 128],
                            idr[:],
                        )
                    et = pet.tile([128, T], f32r)
                    if c < N_EVAC_ACT:
                        nc.scalar.copy(et[:], tp_ps[:])
                    else:
                        nc.vector.tensor_copy(et[:], tp_ps[:])
                    nc.tensor.matmul(
                        kl_ps[:], Wk_s[:, c, :], et[:],
                        start=(c == 0), stop=False,
                    )
                nc.tensor.matmul(kl_ps[:], Mmat_s[:], band_octs[b // 8][:, b % 8, :], start=False, stop=True)

                # tanh(k + loc + (q + cbias))
                tanh_t = ptanh.tile([128, T], f32r)
                nc.scalar.activation(tanh_t[:], kl_ps[:], AF.Tanh, bias=qcb[:, b:b + 1])

                # energies -> PSUM [1, T] @ partition 0
                e_ps = ps_e.tile([1, T], f32, tag="e")
                nc.tensor.matmul(e_ps[:], Wsc_s[:], tanh_t[:], start=True, stop=True)

                # p = exp(e), s = sum(p); w = p / s
                p_row = pmisc.tile([1, T], f32r, tag="prow")
                s_row = pmisc.tile([1, 1], f32, tag="srow")
                nc.scalar.activation(p_row[:], e_ps[:], AF.Exp, accum_out=s_row[:])
                r_row = pmisc.tile([1, 1], f32, tag="rrow")
                nc.vector.reciprocal(r_row[:], s_row[:])
                w_row = pmisc.tile([1, T], f32r, tag="wrow")
                nc.vector.tensor_scalar_mul(w_row[:], p_row[:], r_row[:])
                nc.gpsimd.dma_start(neww_d[b:b + 1, :], w_row[:])

                # p^T chunks [128, NT] (unnormalized; 1/s folded into ctx evac)
                wt_ps = ps_sm.tile([128, NT], f32, tag="sm")
                for t in range(NT):
                    nc.tensor.transpose(
                        wt_ps[:, t:t + 1],
                        p_row[0:1, t * 128:(t + 1) * 128].bitcast(f32),
                        idf[:1, :1],
                    )
                wT_s = pmisc.tile([128, NT], f32r, tag="wT")
                nc.scalar.copy(wT_s[:], wt_ps[:])

                # ctx[1, D_ENC] = sum_t w_t E[t, :]
                ctx_ps0 = ps_ctx.tile([1, D_DEC], f32, tag="ctx")
                ctx_ps1 = ps_ctx.tile([1, D_DEC], f32, tag="ctx")
                for h, cps in enumerate((ctx_ps0, ctx_ps1)):
                    for t in range(NT):
                        nc.tensor.matmul(
                            cps[:],
                            wT_s[:, t:t + 1],
                            e_nat[:, t, h * D_DEC:(h + 1) * D_DEC],
                            start=(t == 0), stop=(t == NT - 1),
                        )
                ctx_s = pmisc.tile([1, 2, D_DEC], f32r, tag="ctxs")
                nc.vector.tensor_scalar_mul(ctx_s[:, 0, :], ctx_ps0[:], r_row[:])
                nc.vector.tensor_scalar_mul(ctx_s[:, 1, :], ctx_ps1[:], r_row[:])

                # ctx^T chunks into per-batch column
                ctT_ps = ps_sm.tile([128, ND], f32, tag="sm")
                for c in range(ND):
                    nc.tensor.transpose(
                        ctT_ps[:, c:c + 1],
                        ctx_s[0:1, c // (D_DEC // 128), (c % (D_DEC // 128)) * 128:(c % (D_DEC // 128) + 1) * 128].bitcast(f32),
                        idf[:1, :1],
                    )
                nc.scalar.copy(ctxT_all[:, :, b], ctT_ps[:])

            # ---------------- postamble ----------------
            fp_ps = ps_tp.tile([b_pc, D_DEC], f32, tag="tp")
            for c in range(ND):
                nc.tensor.matmul(
                    fp_ps[:], ctxT_all[:, c, :], Wv_s[:, c, :],
                    start=(c == 0), stop=(c == ND - 1),
                )
            ctx_out_s = pw.tile([b_pc, D_DEC], f32r)
            nc.scalar.copy(ctx_out_s[:], fp_ps[:])
            nc.sync.dma_start(ctx_d[:], ctx_out_s[:])

    nc.finalize()
    return nc


_NC_CACHE = {}


def _get_nc(b_pc):
    if b_pc not in _NC_CACHE:
        _NC_CACHE[b_pc] = build_nc(b_pc)
    return _NC_CACHE[b_pc]


def kernel(query, encoder_output, attention_weights, Wq, Wk, Wv, Wloc,
           conv_w, conv_b, Wscore, _trace=False, _trace_kwargs=None):
    from concourse.bass_utils import run_bass_kernel_spmd

    b_pc = B // N_CORES
    nc = _get_nc(b_pc)
    shared = {
        "Wq": np.asarray(Wq, dtype=np.float32),
        "Wk": np.asarray(Wk, dtype=np.float32),
        "Wv": np.asarray(Wv, dtype=np.float32),
        "Wloc": np.asarray(Wloc, dtype=np.float32),
        "conv_w": np.asarray(conv_w, dtype=np.float32),
        "conv_b": np.asarray(conv_b, dtype=np.float32),
        "Wscore": np.asarray(Wscore, dtype=np.float32),
    }
    query = np.asarray(query, dtype=np.float32)
    encoder_output = np.asarray(encoder_output, dtype=np.float32)
    attention_weights = np.asarray(attention_weights, dtype=np.float32)
    n_enc_chunks = max(1, b_pc // 4)
    enc_bpc = b_pc // n_enc_chunks
    in_maps = []
    for c in range(N_CORES):
        sl = slice(c * b_pc, (c + 1) * b_pc)
        m = {
            "query": query[sl],
            "attention_weights": attention_weights[sl],
            **shared,
        }
        for i in range(n_enc_chunks):
            lo = c * b_pc + i * enc_bpc
            m[f"encoder_output_{i}"] = encoder_output[lo:lo + enc_bpc]
        in_maps.append(m)
    kw = {}
    if _trace:
        kw = {"trace": True, **(_trace_kwargs or {})}
    res = run_bass_kernel_spmd(nc, in_maps, list(range(N_CORES)), **kw)
    ctx = np.concatenate([res.results[c]["context"] for c in range(N_CORES)], axis=0)
    neww = np.concatenate([res.results[c]["new_w"] for c in range(N_CORES)], axis=0)
    kernel._last_result = res
    return ctx, neww


# revision 18
# speedup vs baseline: 339.6367x; 1.2782x over previous
"""Trainium2 Bass kernel for the location-sensitive attention module.

Math (per batch b):
    q    = query @ Wq                              # (D_att,)
    k    = E @ Wk                                  # (T, D_att)
    loc  = conv1d(aw) -> (F, T);  loc_a = Wloc^T @ (conv + conv_b)
         = sum_k aw_pad[t+k] * M[k, :] + cbias     # M = conv_w^T @ Wloc  (31, 128)
    e_t  = tanh(q + k_t + loc_t) . Wscore          # (T,)
    w    = softmax(e)                              # (T,)
    ctx  = (w @ E) @ Wv                            # (D_dec,)

Sharding: data-parallel over batch across 8 cores (32 batches each).

Per-core dataflow (per batch):
    DMA E_b [512, 1024] natural ->
    PE-transpose 128x128 blocks -> PSUM -> ACT/DVE evac -> E^T chunks [128d, 512t]
    k^T[a,t]  = sum_dchunks Wk_c^T @ E^T_c  (+ loc matmul, accumulated in PSUM)
    tanh via ACT with per-partition bias q+cbias -> energy matvec (PE)
    exp+sum via ACT (accum_out), unnormalized p kept; reciprocal folded in at end
    w^T via small PE transposes -> ctx[1,1024] = sum_t p_t E[t,:]  (PE)
    ctx^T via small PE transposes -> batched final projection (w/ Wv) at end.
"""

import numpy as np

import concourse.bacc as bacc
import concourse.bass as bass
import concourse.mybir as mybir
import concourse.tile as tile
from concourse import masks

f32r = mybir.dt.float32r
f32 = mybir.dt.float32
AF = mybir.ActivationFunctionType

N_CORES = 8
B, T, D_DEC, D_ENC, D_ATT = 256, 512, 512, 1024, 128
N_FILT, KW, PAD = 32, 31, 15
B_PC = B // N_CORES

NT = T // 128          # 4 t-chunks
ND = D_ENC // 128      # 8 d-chunks
NQ = D_DEC // 128      # 4 dec-chunks
N_EVAC_ACT = 5         # d-chunks 0..4 evacuated by ACT, rest by DVE


def build_nc(b_pc=B_PC, bench_loops=1):
    nc = bacc.Bacc(target_bir_lowering=False)

    # encoder input split into chunks: single >16MB buffers wedge the
    # axon PJRT transfer path, so keep each ExternalInput buffer small
    n_enc_chunks = max(1, b_pc // 4)
    enc_chunks = [
        nc.dram_tensor(f"encoder_output_{i}", [b_pc // n_enc_chunks, T, D_ENC],
                       f32r, kind="ExternalInput")
        for i in range(n_enc_chunks)
    ]
    enc_bpc = b_pc // n_enc_chunks
    query = nc.dram_tensor("query", [b_pc, D_DEC], f32r, kind="ExternalInput")
    aw = nc.dram_tensor("attention_weights", [b_pc, T], f32r, kind="ExternalInput")
    Wq = nc.dram_tensor("Wq", [D_DEC, D_ATT], f32r, kind="ExternalInput")
    Wk = nc.dram_tensor("Wk", [D_ENC, D_ATT], f32r, kind="ExternalInput")
    Wv = nc.dram_tensor("Wv", [D_ENC, D_DEC], f32r, kind="ExternalInput")
    Wloc = nc.dram_tensor("Wloc", [N_FILT, D_ATT], f32r, kind="ExternalInput")
    conv_w = nc.dram_tensor("conv_w", [N_FILT, 1, KW], f32r, kind="ExternalInput")
    conv_b = nc.dram_tensor("conv_b", [N_FILT], f32r, kind="ExternalInput")
    Wscore = nc.dram_tensor("Wscore", [D_ATT, 1], f32r, kind="ExternalInput")
    ctx_d = nc.dram_tensor("context", [b_pc, D_DEC], f32r, kind="ExternalOutput")
    neww_d = nc.dram_tensor("new_w", [b_pc, T], f32r, kind="ExternalOutput")

    import contextlib

    with tile.TileContext(nc) as tc:
        loop_cm = tc.For_i(0, bench_loops, 1) if bench_loops > 1 else contextlib.nullcontext()
        with loop_cm:
          with (
            tc.tile_pool(name="pw", bufs=1) as pw,            # persistent weights/state
            tc.tile_pool(name="pnat", bufs=5) as pnat,        # E natural tiles
            tc.tile_pool(name="pet", bufs=8) as pet,          # E^T chunks
            tc.tile_pool(name="ptanh", bufs=3) as ptanh,
            tc.tile_pool(name="pmisc", bufs=3) as pmisc,
            tc.tile_pool(name="pband", bufs=2) as pband,
            tc.tile_pool(name="pdram", bufs=1, space="DRAM") as pdram,
            tc.tile_pool(name="ps_tp", bufs=3, space="PSUM") as ps_tp,
            tc.tile_pool(name="ps_kl", bufs=2, space="PSUM") as ps_kl,
            tc.tile_pool(name="ps_sm", bufs=1, space="PSUM") as ps_sm,
            tc.tile_pool(name="ps_ctx", bufs=2, space="PSUM") as ps_ctx,
        ):
            # ---------------- preamble ----------------
            idf = pw.tile([128, 128], f32)
            masks.make_identity(nc, idf[:])
            idr = pw.tile([128, 128], f32r)
            nc.scalar.copy(idr[:], idf[:])
            dum = pw.tile([1, 128], f32)
            nc.gpsimd.memset(dum[:], 0.0)

            # first encoder tiles before anything else so the PE starts early
            nat_tiles, band_tiles = {}, {}
            band_d = pdram.tile([b_pc, T + 2 * PAD], f32r)

            def issue_nat(b):
                if b >= b_pc or b in nat_tiles:
                    return
                e_nat = pnat.tile([128, NT, D_ENC], f32r)
                src_ap = enc_chunks[b // enc_bpc][b % enc_bpc]
                nc.sync.dma_start(e_nat[:], src_ap.rearrange("(t p) d -> p t d", p=128))
                nat_tiles[b] = e_nat

            def issue_loads(b):
                issue_nat(b)

            issue_nat(0)
            issue_nat(1)

            # padded attention_weights staged once through DRAM; the per-oct
            # band reads use an overlapping AP over the padded rows
            awp_s = pw.tile([b_pc, T + 2 * PAD], f32r)
            nc.vector.memset(awp_s[:].bitcast(mybir.dt.uint32), 0)
            nc.sync.dma_start(awp_s[:, PAD:PAD + T], aw[:])
            nc.sync.dma_start(band_d[:], awp_s[:])
            band_octs = {}
            _stride = T + 2 * PAD

            def issue_oct(i):
                if i * 8 >= b_pc or i in band_octs:
                    return
                n = min(8, b_pc - i * 8)
                boct = pband.tile([KW, 8, T], f32r, tag="boct")
                nc.sync.dma_start(
                    boct[:, :n, :],
                    bass.AP(band_d.tensor, band_d[:].offset + i * 8 * _stride,
                            [[1, KW], [_stride, n], [1, T]]),
                )
                band_octs[i] = boct

            issue_oct(0)

            # weight loads, ordered by first use
            Wk_s = pw.tile([128, ND, D_ATT], f32r)
            nc.sync.dma_start(Wk_s[:], Wk[:].rearrange("(c p) a -> p c a", p=128))
            query_s = pw.tile([b_pc, D_DEC], f32r)
            nc.sync.dma_start(query_s[:], query[:])
            Wq_s = pw.tile([128, NQ, D_ATT], f32r)
            nc.sync.dma_start(Wq_s[:], Wq[:].rearrange("(c p) a -> p c a", p=128))
            Wloc_s = pw.tile([N_FILT, D_ATT], f32r)
            nc.sync.dma_start(Wloc_s[:], Wloc[:])
            convw_s = pw.tile([N_FILT, KW], f32r)
            nc.sync.dma_start(convw_s[:], conv_w[:, 0, :])
            convb_s = pw.tile([N_FILT, 2], f32r)
            nc.vector.memset(convb_s[:].bitcast(mybir.dt.uint32), 0)
            nc.sync.dma_start(convb_s[:, 0:1], bass.AP(conv_b, 0, [[1, N_FILT], [1, 1]]))
            Wsc_s = pw.tile([D_ATT, 1], f32r)
            nc.sync.dma_start(Wsc_s[:], Wscore[:])

            Wv_s = pw.tile([128, ND, D_DEC], f32r)
            nc.sync.dma_start(Wv_s[:], Wv[:].rearrange("(c p) a -> p c a", p=128))

            # PE warmup: absorb gpsimd tick
            dum_ps = ps_sm.tile([128, 128], f32, tag="sm")
            nc.tensor.transpose(dum_ps[:, :1], dum[:], idf[:1, :1])

            # q^T: transpose query then project:  qT[a, b] = sum_dec Wq[dec, a] query[b, dec]^T
            qtr_ps = ps_sm.tile([128, NQ * b_pc], f32r, tag="sm")
            for c in range(NQ):
                nc.tensor.transpose(
                    qtr_ps[:, c * b_pc:(c + 1) * b_pc],
                    query_s[:, c * 128:(c + 1) * 128],
                    idr[:b_pc, :b_pc],
                )
            qT_s = pw.tile([128, NQ, b_pc], f32r)
            nc.scalar.copy(qT_s[:].rearrange("p c b -> p (c b)"), qtr_ps[:])
            qt_ps = ps_kl.tile([128, b_pc], f32, tag="kl")
            for c in range(NQ):
                nc.tensor.matmul(
                    qt_ps[:], Wq_s[:, c, :], qT_s[:, c, :],
                    start=(c == 0), stop=(c == NQ - 1),
                )

            # cbias^T[a] = sum_f Wloc[f, a] conv_b[f]
            cb_ps = ps_sm.tile([128, 2], f32, tag="sm")
            nc.tensor.matmul(cb_ps[:], Wloc_s[:], convb_s[:], start=True, stop=True)
            cb_s = pw.tile([128, 1], f32)
            nc.scalar.copy(cb_s[:], cb_ps[:, 0:1])

            # M[k, a] = sum_f conv_w[f, k] Wloc[f, a]
            mm_ps = ps_sm.tile([KW, D_ATT], f32, tag="sm")
            nc.tensor.matmul(mm_ps[:], convw_s[:], Wloc_s[:], start=True, stop=True)
            Mmat_s = pw.tile([KW, D_ATT], f32r)
            nc.scalar.copy(Mmat_s[:], mm_ps[:])

            # qcb[a, b] = qT + cbias  (tanh bias, per-partition over a)
            qcb = pw.tile([128, b_pc], f32)
            nc.vector.tensor_scalar_add(qcb[:], qt_ps[:], cb_s[:])

            # persistent state
            ctxT_all = pw.tile([128, ND, b_pc], f32r)

            # ---------------- main loop ----------------
            for b in range(b_pc):
                issue_loads(b + 2)
                if b % 8 == 2:
                    issue_oct(b // 8 + 1)
                e_nat = nat_tiles.pop(b)

                kl_ps = ps_kl.tile([128, T], f32, tag="kl")
                for c in range(ND):
                    tp_ps = ps_tp.tile([128, T], f32r, tag="tp")
                    for t in range(NT):
                        nc.tensor.transpose(
                            tp_ps[:, t * 128:(t + 1) * 128],
                            e_nat[:, t, c * 128:(c + 1) * 128],
                            idr[:],
                        )
                    et = pet.tile([128, T], f32r)
                    if c % 2 == 0:
                        nc.scalar.copy(et[:], tp_ps[:])
                    else:
                        nc.vector.tensor_copy(et[:], tp_ps[:])
                    nc.tensor.matmul(
                        kl_ps[:], Wk_s[:, c, :], et[:],
                        start=(c == 0), stop=False,
                    )
                nc.tensor.matmul(kl_ps[:], Mmat_s[:], band_octs[b // 8][:, b % 8, :], start=False, stop=True)

                # tanh(k + loc + (q + cbias))
                tanh_t = ptanh.tile([128, T], f32r)
                nc.scalar.activation(tanh_t[:], kl_ps[:], AF.Tanh, bias=qcb[:, b:b + 1])

                # energies -> PSUM [1, T] @ partition 0
                e_ps = ps_sm.tile([1, T], f32, tag="sm")
                nc.tensor.matmul(e_ps[:], Wsc_s[:], tanh_t[:], start=True, stop=True)

                # p = exp(e), s = sum(p); w = p / s
                p_row = pmisc.tile([1, T], f32r, tag="prow")
                s_row = pmisc.tile([1, 1], f32, tag="srow")
                nc.scalar.activation(p_row[:], e_ps[:], AF.Exp, accum_out=s_row[:])
                r_row = pmisc.tile([1, 1], f32, tag="rrow")
                nc.vector.reciprocal(r_row[:], s_row[:])
                w_row = pmisc.tile([1, T], f32r, tag="wrow")
                nc.vector.tensor_scalar_mul(w_row[:], p_row[:], r_row[:])
                nc.gpsimd.dma_start(neww_d[b:b + 1, :], w_row[:])

                # p^T chunks [128, NT] (unnormalized; 1/s folded into ctx evac)
                wt_ps = ps_sm.tile([128, NT], f32, tag="sm")
                for t in range(NT):
                    nc.tensor.transpose(
                        wt_ps[:, t:t + 1],
                        p_row[0:1, t * 128:(t + 1) * 128].bitcast(f32),
                        idf[:1, :1],
                    )
                wT_s = pmisc.tile([128, NT], f32r, tag="wT")
                nc.scalar.copy(wT_s[:], wt_ps[:])

                # ctx[1, D_ENC] = sum_t w_t E[t, :]
                ctx_ps0 = ps_ctx.tile([1, D_DEC], f32, tag="ctx")
                ctx_ps1 = ps_ctx.tile([1, D_DEC], f32, tag="ctx")
                for h, cps in enumerate((ctx_ps0, ctx_ps1)):
                    for t in range(NT):
                        nc.tensor.matmul(
                            cps[:],
                            wT_s[:, t:t + 1],
                            e_nat[:, t, h * D_DEC:(h + 1) * D_DEC],
                            start=(t == 0), stop=(t == NT - 1),
                        )
                ctx_s = pmisc.tile([1, 2, D_DEC], f32r, tag="ctxs")
                nc.vector.tensor_scalar_mul(ctx_s[:, 0, :], ctx_ps0[:], r_row[:])
                nc.vector.tensor_scalar_mul(ctx_s[:, 1, :], ctx_ps1[:], r_row[:])

                # ctx^T chunks into per-batch column
                ctT_ps = ps_sm.tile([128, ND], f32, tag="sm")
                for c in range(ND):
                    nc.tensor.transpose(
                        ctT_ps[:, c:c + 1],
                        ctx_s[0:1, c // (D_DEC // 128), (c % (D_DEC // 128)) * 128:(c % (D_DEC // 128) + 1) * 128].bitcast(f32),
                        idf[:1, :1],
                    )
                nc.scalar.copy(ctxT_all[:, :, b], ctT_ps[:])

            # ---------------- postamble ----------------
            fp_ps = ps_tp.tile([b_pc, D_DEC], f32, tag="tp")
            for c in range(ND):
                nc.tensor.matmul(
                    fp_ps[:], ctxT_all[:, c, :], Wv_s[:, c, :],
                    start=(c == 0), stop=(c == ND - 1),
                )
            ctx_out_s = pw.tile([b_pc, D_DEC], f32r)
            nc.scalar.copy(ctx_out_s[:], fp_ps[:])
            nc.sync.dma_start(ctx_d[:], ctx_out_s[:])

    nc.finalize()
    return nc


_NC_CACHE = {}


def _get_nc(b_pc):
    if b_pc not in _NC_CACHE:
        _NC_CACHE[b_pc] = build_nc(b_pc)
    return _NC_CACHE[b_pc]


def kernel(query, encoder_output, attention_weights, Wq, Wk, Wv, Wloc,
           conv_w, conv_b, Wscore, _trace=False, _trace_kwargs=None):
    from concourse.bass_utils import run_bass_kernel_spmd

    b_pc = B // N_CORES
    nc = _get_nc(b_pc)
    shared = {
        "Wq": np.asarray(Wq, dtype=np.float32),
        "Wk": np.asarray(Wk, dtype=np.float32),
        "Wv": np.asarray(Wv, dtype=np.float32),
        "Wloc": np.asarray(Wloc, dtype=np.float32),
        "conv_w": np.asarray(conv_w, dtype=np.float32),
        "conv_b": np.asarray(conv_b, dtype=np.float32),
        "Wscore": np.asarray(Wscore, dtype=np.float32),
    }
    query = np.asarray(query, dtype=np.float32)
    encoder_output = np.asarray(encoder_output, dtype=np.float32)
    attention_weights = np.asarray(attention_weights, dtype=np.float32)
    n_enc_chunks = max(1, b_pc // 4)
    enc_bpc = b_pc // n_enc_chunks
    in_maps = []
    for c in range(N_CORES):
        sl = slice(c * b_pc, (c + 1) * b_pc)
        m = {
            "query": query[sl],
            "attention_weights": attention_weights[sl],
            **shared,
        }
        for i in range(n_enc_chunks):
            lo = c * b_pc + i * enc_bpc
            m[f"encoder_output_{i}"] = encoder_output[lo:lo + enc_bpc]
        in_maps.append(m)
    kw = {}
    if _trace:
        kw = {"trace": True, **(_trace_kwargs or {})}
    res = run_bass_kernel_spmd(nc, in_maps, list(range(N_CORES)), **kw)
    ctx = np.concatenate([res.results[c]["context"] for c in range(N_CORES)], axis=0)
    neww = np.concatenate([res.results[c]["new_w"] for c in range(N_CORES)], axis=0)
    kernel._last_result = res
    return ctx, neww
